# revision 33
# baseline (speedup 1.0000x reference)
"""Trainium2 Bass kernel for the CNN-TRX few-shot attention head.

Sharding: data-parallel over the 200 queries (25 per NeuronCore); support set
and weights replicated per core. v5: fp8(e4m3) DoubleRow matmuls everywhere,
PE-side V-path tuple gather via selection matmuls, natural-layout distances.

  1. Frame projections fp8 DoubleRow (contraction 2048 = 8 chunk pairs),
     K blocks first so the K-side DVE chain starts early; weights
     pre-scaled by 1024; K bias via per-partition activation bias; V bias
     dropped (it cancels exactly in ||q_v - proto||^2).
  2. K tuple gather (C(8,3)=56 triples) as 2-stage DVE column adds;
     V tuple gather on the PE: transpose frame projections to natural
     layout, then Sel.T @ frames with DoubleRow pairing tuple positions
     j0,j1 (+ plain j2) -- produces s_v / q_v directly in natural layout.
  3. Column LayerNorm of K: stats via 2-slot packed ones-matmuls, DVE
     apply writes fp8 (scale 4).
  4. scoresT = s_k^T q_k fp8 (4 DR pairs + 1 plain); exp with -ln(8) bias.
  5. P_nat[row,d] = exp^T s_v per class (DR row-chunk pairs); the s_v ones
     column gives S free; dist rows = ACT square-accum of (P*r - q_v) with
     r per-partition; logits via an indicator matmul.
"""

import math
from itertools import combinations

import ml_dtypes
import numpy as np

SEQ = 8
IN_DIM = 2048
OUT_DIM = 1152
TSS = 3
WAY = 5
N_SUPPORT = 25
N_QUERIES = 200
PE_SCALE = 0.1
LN_EPS = 1e-5
T = 56
N_CORES = 8
NQL = N_QUERIES // N_CORES      # queries per core (25)
G_SIZES = [8, 8, 9]             # query group sizes (sum = NQL)
G_MAX = max(G_SIZES)
C_ALLOC = 512
NKCH = IN_DIM // 128            # 16 contraction chunks
NDCH = OUT_DIM // 128           # 9
NMB = 6 * NDCH                  # 54 projection blocks
NX = SEQ * 2 * N_SUPPORT        # 400 frame columns per core
NFC = (NX + 127) // 128         # 4 frame-row chunks (natural layout)
PAIRS = [(t0, t1) for t0 in range(SEQ - 2) for t1 in range(t0 + 1, SEQ - 1)]
TUPLES = list(combinations(range(SEQ), TSS))
LN_CHUNK = 512
S_W = 1024.0                    # weight fp8 scale
S_K = 4.0                       # LN'd K fp8 scale
S_V = 4.0                       # V fp8 scale
EXP_SHIFT = math.log(8.0)       # exp output scale 1/8 (fp8 range)
SV_W = OUT_DIM + 16             # s_v width: 1152 d + ones col + pad
NSPL = [(0, 512), (512, 512), (1024, OUT_DIM - 1024)]
BF16 = ml_dtypes.bfloat16
F8 = ml_dtypes.float8_e4m3

_CACHE = {}


def _pos_encoding():
    pos = np.arange(SEQ, dtype=np.float32)[:, None]
    div = np.exp(np.arange(0, IN_DIM, 2, dtype=np.float32) * -(math.log(10000.0) / IN_DIM))
    pe = np.zeros((SEQ, IN_DIM), dtype=np.float32)
    pe[:, 0::2] = np.sin(pos * div) * PE_SCALE
    pe[:, 1::2] = np.cos(pos * div) * PE_SCALE
    return pe


def _class_layout(counts):
    offs, off = [], 0
    for c in range(WAY):
        offs.append(off)
        off += ((counts[c] * T + 127) // 128) * 128
    return offs, off


def _sel_pairs(counts):
    """Selection matrices for the PE-side V gather.

    Returns (sel_list, sup_chunks, qry_chunks):
      sel_list: list of [128, 3, 128] float32 0/1 matrices
      sup_chunks[tc] / qry_chunks[(g, tc)]: list of (fc, sel_index)
    Sel[frame_local_row, j, col] = 1 iff dst row (128*tc+col) is tuple a of
    item n and 8*n + TUPLES[a][j] == 128*fc + frame_local_row.
    """
    offs, nb_pad = _class_layout(counts)
    sel_list = []
    sup_chunks = {}

    def build(rows_of):  # rows_of: col -> (item n, tuple a) or None
        chunks = {}
        for col, na in rows_of.items():
            if na is None:
                continue
            n, a = na
            tc = col // 128
            for j in range(TSS):
                fr = 8 * n + TUPLES[a][j]
                fc = fr // 128
                key = (tc, fc)
                if key not in chunks:
                    chunks[key] = np.zeros((128, 3, 128), np.float32)
                chunks[key][fr - 128 * fc, j, col - 128 * tc] = 1.0
        out = {}
        for (tc, fc), mat in sorted(chunks.items()):
            sel_list.append(mat)
            out.setdefault(tc, []).append((fc, len(sel_list) - 1))
        return out

    rows = {}
    for c in range(WAY):
        n0 = int(np.sum(counts[:c]))
        for i in range(int(counts[c]) * T):
            rows[offs[c] + i] = (n0 + i // T, i % T)
    sup_chunks = build(rows)

    qry_chunks = {}
    it0 = N_SUPPORT
    for g, G in enumerate(G_SIZES):
        rows = {i: (it0 + i // T, i % T) for i in range(G * T)}
        for tc, lst in build(rows).items():
            qry_chunks[(g, tc)] = lst
        it0 += G
    return sel_list, sup_chunks, qry_chunks


def _build_kernel(counts, trivial_gb):
    import concourse.mybir as mybir
    import concourse.tile as tile
    from concourse import bacc
    from concourse.masks import make_identity

    f32 = mybir.dt.float32
    bf16 = mybir.dt.bfloat16
    fp8 = mybir.dt.float8e4
    AF = mybir.ActivationFunctionType
    ALU = mybir.AluOpType
    DR = mybir.MatmulPerfMode.DoubleRow
    offs, nb_pad = _class_layout(counts)
    nwch = nb_pad // 128
    inv_sqrt = 1.0 / math.sqrt(OUT_DIM)
    sel_list, sup_chunks, qry_chunks = _sel_pairs(counts)
    nsel = len(sel_list)

    nc = bacc.Bacc("TRN2", target_bir_lowering=False, debug=False,
                   enable_asserts=False, num_devices=N_CORES)

    x_d = nc.dram_tensor("x", [128, NKCH, NX], fp8, kind="ExternalInput").ap()
    w_d = nc.dram_tensor("w", [128, NMB, NKCH, 128], fp8, kind="ExternalInput").ap()
    bias_d = nc.dram_tensor("bias", [128, NMB], f32, kind="ExternalInput").ap()
    sel_d = nc.dram_tensor("sel", [128, nsel, 3, 128], fp8, kind="ExternalInput").ap()
    qind_d = nc.dram_tensor("qind", [128, 4, G_MAX], f32, kind="ExternalInput").ap()
    g_d = nc.dram_tensor("lng", [128, NDCH], bf16, kind="ExternalInput").ap()
    b_d = nc.dram_tensor("lnb", [128, NDCH], bf16, kind="ExternalInput").ap()
    out_d = nc.dram_tensor("out", [NQL, WAY], f32, kind="ExternalOutput").ap()

    with tile.TileContext(nc) as tc:
        with tc.tile_pool(name="big", bufs=1) as big, \
             tc.tile_pool(name="small", bufs=1) as small:
            # frame projections, fp8: K in T-layout (per tuple position j),
            # V transposed to natural layout nfv[frame_row, j, fc, d]
            f_k = [big.tile([128, NDCH, NX], fp8, name=f"fk{j}") for j in range(3)]
            nfv = big.tile([128, 3, NFC, SV_W], fp8, name="nfv")
            kch_c = [((int(counts[c]) * T + 127) // 128) for c in range(WAY)]
            s_kT_c = [big.tile([128, NDCH, kch_c[c] * 128], fp8, name=f"skT{c}")
                      for c in range(WAY)]              # LN'd support K * S_K
            s_v = big.tile([128, nwch, SV_W], fp8)          # support V * S_V, natural
            ones_bf = small.tile([128, 1], bf16)
            nc.vector.memset(ones_bf, 1.0)
            eps_sb = small.tile([1, 1], f32)
            nc.vector.memset(eps_sb, LN_EPS)
            expb_sb = small.tile([128, 1], f32)
            nc.vector.memset(expb_sb, -EXP_SHIFT)
            g_sb = small.tile([128, NDCH], bf16)
            b_sb = small.tile([128, NDCH], bf16)
            bias_sb = small.tile([128, NMB], f32)
            qind_sb = small.tile([128, 4, G_MAX], f32)
            nc.sync.dma_start(g_sb, g_d)
            nc.sync.dma_start(b_sb, b_d)
            nc.sync.dma_start(bias_sb, bias_d)
            nc.sync.dma_start(qind_sb, qind_d)
            logits5 = small.tile([WAY, NQL], f32)
            ident = small.tile([128, 128], bf16)
            make_identity(nc, ident)
            # last frame chunk holds only 16 rows -> zero the tail once
            nc.gpsimd.memset(nfv[:, :, NFC - 1, :], 0.0)

            # ---------- Phase 1: frame projections (fp8 DoubleRow) ----------
            # K blocks (m = j*9+dd) first, then V blocks (m = 27 + j*9+dd)
            sprep_cm = tc.tile_pool(name="sprep", bufs=2)
            pp_t_cm = tc.tile_pool(name="pp_t", bufs=4, space="PSUM")
            sprep = sprep_cm.__enter__()
            pp_t = pp_t_cm.__enter__()
            with tc.tile_pool(name="xt_pool", bufs=1) as xt_pool, \
                 tc.tile_pool(name="xw", bufs=3) as xw, \
                 tc.tile_pool(name="pp_proj", bufs=4, space="PSUM") as pp_proj:
                xt = xt_pool.tile([128, NKCH, NX], fp8)
                nc.sync.dma_start(xt, x_d)
                fv_ps = []          # V-block PSUM tiles, consumed by transposes
                for m in range(NMB):
                    kv, j, dd = m // 27, (m % 27) // 9, m % 9
                    wm = xw.tile([128, NKCH, 128], fp8, tag="wslab")
                    nc.sync.dma_start(wm, w_d[:, m])
                    ps = pp_proj.tile([128, NX], f32, tag="projps")
                    for k in range(NKCH // 2):
                        nc.tensor.matmul(ps, wm[:, 2 * k:2 * k + 2],
                                         xt[:, 2 * k:2 * k + 2],
                                         start=(k == 0), stop=(k == NKCH // 2 - 1),
                                         perf_mode=DR)
                    if kv == 0:
                        nc.scalar.activation(f_k[j][:, dd], ps, AF.Identity,
                                             bias=bias_sb[:, m:m + 1], scale=1.0 / S_W)
                    else:
                        # V: bf16 T-layout scratch (scaled S_V), then transpose
                        fvt = sprep.tile([128, NX], bf16, tag="fvt", name="fvt",
                                         bufs=4)
                        nc.scalar.activation(fvt, ps, AF.Copy, scale=S_V / S_W)
                        for fc in range(NFC):
                            cw = min(128, NX - fc * 128)
                            pst = pp_t.tile([128, 128], bf16, tag="tps")
                            nc.tensor.transpose(
                                pst[:cw], fvt[:, fc * 128:fc * 128 + cw], ident)
                            if dd % 2 == 0:
                                nc.vector.tensor_copy(
                                    nfv[:cw, j, fc, dd * 128:(dd + 1) * 128], pst[:cw])
                            else:
                                nc.scalar.activation(
                                    nfv[:cw, j, fc, dd * 128:(dd + 1) * 128],
                                    pst[:cw], AF.Copy)

            f_i = [fk.rearrange("p b (i s) -> p b i s", s=SEQ) for fk in f_k]

            def gather_k(dst4, items0, n_items, pool):
                """dst4 [128, 9, n_items, T] = tuple-gathered K projections."""
                isl = slice(items0, items0 + n_items)
                b0, b1, b2 = f_i
                p2 = pool.tile([128, NDCH, n_items, len(PAIRS)], bf16,
                               tag="pairs", name="p2")
                pi = 0
                for t0 in range(SEQ - 2):
                    run = SEQ - 2 - t0
                    a = b0[:, :, isl, t0:t0 + 1]
                    b = b1[:, :, isl, t0 + 1:t0 + 1 + run]
                    nc.vector.tensor_add(p2[:, :, :, pi:pi + run],
                                         a.to_broadcast(b.shape), b)
                    pi += run
                ai = 0
                for pi, (t0, t1) in enumerate(PAIRS):
                    run = SEQ - 1 - t1
                    a = p2[:, :, :, pi:pi + 1]
                    b = b2[:, :, isl, t1 + 1:t1 + 1 + run]
                    nc.vector.tensor_add(dst4[:, :, :, ai:ai + run],
                                         a.to_broadcast(b.shape), b)
                    ai += run

            def col_ln(raw, cols, out_q, pool, psum_pool):
                """Column-wise LayerNorm of raw [128, NDCH, cols] bf16
                (in place); writes fp8 out_q = LN(x)*S_K."""
                for c0 in range(0, cols, LN_CHUNK):
                    cw = min(LN_CHUNK, cols - c0)
                    r = raw[:, :, c0:c0 + cw]
                    o = out_q[:, :, c0:c0 + cw]
                    sq = pool.tile([128, NDCH, cw], bf16, tag="lnsq", name="lnsq", bufs=1)
                    nc.vector.tensor_mul(sq, r, r)
                    ps2 = psum_pool.tile([64, cw], f32, tag="lnps", name="lnps", bufs=1)
                    for k in range(NDCH):
                        nc.tensor.matmul(ps2[0:1], ones_bf, r[:, k],
                                         start=(k == 0), stop=(k == NDCH - 1),
                                         tile_position=(0, 0), skip_group_check=True)
                        nc.tensor.matmul(ps2[32:33], ones_bf, sq[:, k],
                                         start=(k == 0), stop=(k == NDCH - 1),
                                         tile_position=(0, 32), skip_group_check=True)
                    m_r = pool.tile([1, cw], f32, tag="lnm", name="lnm")
                    v_r = pool.tile([1, cw], f32, tag="lnv", name="lnv")
                    mm = pool.tile([1, cw], f32, tag="lnmm", name="lnmm")
                    nc.scalar.activation(m_r, ps2[0:1], AF.Copy, scale=1.0 / OUT_DIM)
                    nc.scalar.activation(v_r, ps2[32:33], AF.Copy, scale=1.0 / OUT_DIM)
                    nc.vector.tensor_mul(mm, m_r, m_r)
                    nc.vector.tensor_sub(v_r, v_r, mm)
                    nc.scalar.activation(v_r, v_r, AF.Sqrt, bias=eps_sb)
                    nc.vector.reciprocal(v_r, v_r)
                    m_h = pool.tile([1, cw], bf16, tag="lnmh", name="lnmh")
                    v_h = pool.tile([1, cw], bf16, tag="lnvh", name="lnvh")
                    nc.vector.tensor_copy(m_h, m_r)
                    if trivial_gb:
                        nc.vector.tensor_scalar(v_h, v_r, S_K, None, ALU.mult)
                    else:
                        nc.vector.tensor_copy(v_h, v_r)
                    m_b = pool.tile([128, cw], bf16, tag="lnmb", name="lnmb", bufs=1)
                    a_b = pool.tile([128, cw], bf16, tag="lnab", name="lnab", bufs=1)
                    nc.gpsimd.partition_broadcast(m_b, m_h)
                    nc.gpsimd.partition_broadcast(a_b, v_h)
                    mb3 = m_b[:, None, :].to_broadcast([128, NDCH, cw])
                    ab3 = a_b[:, None, :].to_broadcast([128, NDCH, cw])
                    nc.vector.tensor_sub(r, r, mb3)
                    if trivial_gb:
                        nc.vector.tensor_mul(o, r, ab3)
                    else:
                        nc.vector.tensor_mul(r, r, ab3)
                        for k in range(NDCH):
                            nc.vector.tensor_scalar(o[:, k], r[:, k],
                                                    g_sb[:, k:k + 1], b_sb[:, k:k + 1],
                                                    ALU.mult, ALU.add)

            def sel_gather(chunk_pairs, dst_fn, selpool, pspool, pstag):
                """dst[tc] rows = sum_j Sel_j^T nfv_j (tuple gather on PE)."""
                for tc, lst in sorted(chunk_pairs.items()):
                    sels = []
                    for fc, si in lst:
                        ssb = selpool.tile([128, 3, 128], fp8, tag="sel", name="sel")
                        nc.sync.dma_start(ssb, sel_d[:, si])
                        sels.append((fc, ssb))
                    for ni, (nlo, nw) in enumerate(NSPL):
                        ps = pspool.tile([128, 512], f32, tag=pstag)
                        for i, (fc, ssb) in enumerate(sels):
                            nc.tensor.matmul(ps[:, :nw], ssb[:, 0:2],
                                             nfv[:, 0:2, fc, nlo:nlo + nw],
                                             start=(i == 0), stop=False,
                                             perf_mode=DR)
                            nc.tensor.matmul(ps[:, :nw], ssb[:, 2],
                                             nfv[:, 2, fc, nlo:nlo + nw],
                                             start=False, stop=(i == len(sels) - 1))
                        dst_fn(tc, ni, nlo, nw, ps)

            # ---------- Phase 2: support-side tensors ----------
            max_ch = max((int(counts[c]) * T + 127) // 128 for c in range(WAY))
            with tc.tile_pool(name="pp_s", bufs=2, space="PSUM") as pp_s:
                start_item = 0
                for c in range(WAY):
                    n_c = int(counts[c])
                    rows = n_c * T
                    scratch = sprep.tile([128, NDCH, max_ch * 128], bf16,
                                         tag="skv", name="skv")
                    dst4 = scratch[:, :, :rows].rearrange("p b (n a) -> p b n a", a=T)
                    gather_k(dst4, start_item, n_c, sprep)
                    if rows < kch_c[c] * 128:
                        nc.gpsimd.memset(s_kT_c[c][:, :, rows:], 0.0)
                    col_ln(scratch[:, :, :rows], rows,
                           s_kT_c[c][:, :, :rows], sprep, pp_s)
                    start_item += n_c
                # s_v ones col: 1 on real rows, 0 on pad rows
                for c in range(WAY):
                    rows = int(counts[c]) * T
                    wlo = offs[c] // 128
                    for w in range((rows + 127) // 128):
                        kreal = min(128, rows - w * 128)
                        if kreal < 128:
                            nc.gpsimd.memset(s_v[:, wlo + w, OUT_DIM:OUT_DIM + 1], 0.0)
                        nc.gpsimd.memset(s_v[:kreal, wlo + w, OUT_DIM:OUT_DIM + 1], 1.0)

                def sv_dst(tc, ni, nlo, nw, ps):
                    if ni % 2 == 0:
                        nc.scalar.activation(s_v[:, tc, nlo:nlo + nw], ps[:, :nw],
                                             AF.Copy)
                    else:
                        nc.vector.tensor_copy(s_v[:, tc, nlo:nlo + nw], ps[:, :nw])
                sel_gather(sup_chunks, sv_dst, sprep, pp_s, "selps")
            pp_t_cm.__exit__(None, None, None)
            sprep_cm.__exit__(None, None, None)

            # ---------- Phase 3: per-group query pipeline ----------
            with tc.tile_pool(name="grp", bufs=2) as grp, \
                 tc.tile_pool(name="rows", bufs=2) as rows_pool, \
                 tc.tile_pool(name="pp_sc", bufs=2, space="PSUM") as pp_sc, \
                 tc.tile_pool(name="pp_pr", bufs=3, space="PSUM") as pp_pr, \
                 tc.tile_pool(name="pp_row", bufs=2, space="PSUM") as pp_row:
                items0 = N_SUPPORT
                q_off = 0
                for g, G in enumerate(G_SIZES):
                    C = G * T
                    nmch = (C + 127) // 128
                    scr = grp.tile([128, NDCH, G_MAX, T], bf16, tag="qkv")
                    gather_k(scr[:, :, :G], items0, G, grp)
                    qk_raw = scr[:, :, :G].rearrange("p m q a -> p m (q a)")
                    qk3 = grp.tile([128, NDCH, C_ALLOC], fp8, tag="qk8")
                    col_ln(qk_raw, C, qk3[:, :, :C], grp, pp_row)

                    # q_v natural via PE sel-gather
                    qv_nat = grp.tile([128, 4, OUT_DIM], bf16, tag="qvn")

                    def qv_dst(tc, ni, nlo, nw, ps):
                        if ni % 2 == 0:
                            nc.vector.tensor_copy(qv_nat[:, tc, nlo:nlo + nw],
                                                  ps[:, :nw])
                        else:
                            nc.scalar.activation(qv_nat[:, tc, nlo:nlo + nw],
                                                 ps[:, :nw], AF.Copy)
                    sel_gather({tc: lst for (gg, tc), lst in qry_chunks.items()
                                if gg == g}, qv_dst, grp, pp_sc, "scps")

                    # scoresT (fp8: 4 DoubleRow pairs + 1 plain) + exp
                    exp_t = grp.tile([128, nwch, C_ALLOC], fp8, tag="exp")
                    wmap = [(c, wi) for c in range(WAY) for wi in range(kch_c[c])]
                    for w, (wc, wi) in enumerate(wmap):
                        skc = s_kT_c[wc]
                        ps = pp_sc.tile([128, C_ALLOC], f32, tag="scps")
                        for k in range(4):
                            nc.tensor.matmul(ps[:, :C],
                                             skc[:, 2 * k:2 * k + 2, wi * 128:(wi + 1) * 128],
                                             qk3[:, 2 * k:2 * k + 2, :C],
                                             start=(k == 0), stop=False, perf_mode=DR)
                        nc.tensor.matmul(ps[:, :C], skc[:, 8, wi * 128:(wi + 1) * 128],
                                         qk3[:, 8, :C], start=False, stop=True)
                        nc.scalar.activation(exp_t[:, w, :C], ps[:, :C], AF.Exp,
                                             scale=inv_sqrt / (S_K * S_K),
                                             bias=expb_sb)

                    dist5 = grp.tile([128, 4 * WAY], f32, tag="dist5")
                    nc.gpsimd.memset(dist5, 0.0)
                    scr_nat = grp.tile([128, OUT_DIM], bf16, tag="scrn")
                    ps_l = pp_row.tile([WAY, G_MAX], f32, tag="psl", bufs=1)
                    for c in range(WAY):
                        rows = int(counts[c]) * T
                        wlo = offs[c] // 128
                        nw_c = (rows + 127) // 128
                        np_pairs = nw_c // 2
                        for mc in range(nmch):
                            mw = min(128, C - mc * 128)
                            msl = slice(mc * 128, mc * 128 + mw)
                            psn = []
                            for ni in (2, 0, 1):
                                nlo, nw = NSPL[ni]
                                ps_p = pp_pr.tile([128, 512], f32, tag="prps")
                                psn.append((ni, ps_p))
                                nhi = nlo + nw + (1 if ni == 2 else 0)  # + S col
                                for wi in range(np_pairs):
                                    nc.tensor.matmul(
                                        ps_p[:mw, :nhi - nlo],
                                        exp_t[:, wlo + 2 * wi:wlo + 2 * wi + 2, msl],
                                        s_v[:, wlo + 2 * wi:wlo + 2 * wi + 2, nlo:nhi],
                                        start=(wi == 0),
                                        stop=(wi == np_pairs - 1 and nw_c % 2 == 0),
                                        perf_mode=DR)
                                if nw_c % 2 == 1:
                                    nc.tensor.matmul(
                                        ps_p[:mw, :nhi - nlo],
                                        exp_t[:, wlo + nw_c - 1, msl],
                                        s_v[:, wlo + nw_c - 1, nlo:nhi],
                                        start=(nw_c == 1), stop=True)
                                if ni == 2:
                                    rr = rows_pool.tile([128, 1], f32, tag="rr")
                                    nc.vector.reciprocal(
                                        rr[:mw], ps_p[:mw, NSPL[2][1]:NSPL[2][1] + 1])
                            # diff = P*r - q_v (both at 4x true scale)
                            for ni, ps_p in psn:
                                nlo, nw = NSPL[ni]
                                nc.vector.scalar_tensor_tensor(
                                    scr_nat[:mw, nlo:nlo + nw], ps_p[:mw, :nw],
                                    rr[:mw], qv_nat[:mw, mc, nlo:nlo + nw],
                                    ALU.mult, ALU.subtract)
                            nc.scalar.activation(
                                scr_nat[:mw], scr_nat[:mw], AF.Square,
                                accum_out=dist5[:mw, 4 * c + mc:4 * c + mc + 1])
                    # logits: indicator matmul sums dist rows per query
                    d54 = dist5.rearrange("p (c m) -> p c m", m=4)
                    for mc in range(nmch):
                        nc.tensor.matmul(ps_l[:, :G], d54[:, :, mc],
                                         qind_sb[:, mc, :G],
                                         start=(mc == 0), stop=(mc == nmch - 1))
                    nc.scalar.activation(logits5[:, q_off:q_off + G], ps_l[:, :G],
                                         AF.Copy, scale=-1.0 / (T * S_V * S_V))
                    items0 += G
                    q_off += G

            nc.sync.dma_start(out_d.rearrange("q c -> c q"), logits5)

    nc.compile()
    return nc


def kernel(support_set, support_labels, queries, k_w, k_b, v_w, v_b, ln_g, ln_b):
    import concourse.bass_utils as bass_utils

    support_set = np.asarray(support_set, dtype=np.float32)
    queries = np.asarray(queries, dtype=np.float32)
    labels = np.asarray(support_labels, dtype=np.int32)
    k_w = np.asarray(k_w, dtype=np.float32)
    v_w = np.asarray(v_w, dtype=np.float32)
    k_b = np.asarray(k_b, dtype=np.float32)
    v_b = np.asarray(v_b, dtype=np.float32)
    ln_g = np.asarray(ln_g, dtype=np.float32)
    ln_b = np.asarray(ln_b, dtype=np.float32)

    pe = _pos_encoding()
    s = support_set + pe[None]
    q = queries + pe[None]
    order = np.argsort(labels, kind="stable")
    counts = np.bincount(labels, minlength=WAY)
    s_sorted = s[order]
    trivial_gb = bool(np.all(ln_g == 1.0) and np.all(ln_b == 0.0))

    key = (tuple(int(x) for x in counts), trivial_gb)
    if key not in _CACHE:
        _CACHE[key] = _build_kernel(counts, trivial_gb)
    nc = _CACHE[key]

    def to_f8(x):
        return np.clip(x, -240.0, 240.0).astype(F8)

    # K blocks m = j*9+dd, V blocks m = 27 + j*9+dd; scaled by S_W
    W = np.zeros((128, NMB, NKCH, 128), np.float32)
    bias = np.zeros((128, NMB), np.float32)
    for kv, (wsrc, bsrc) in enumerate(((k_w, k_b), (v_w, v_b))):
        for j in range(TSS):
            blk = wsrc[j * IN_DIM:(j + 1) * IN_DIM] * S_W   # [2048, 1152]
            blk = blk.reshape(NKCH, 128, NDCH, 128)
            for dd in range(NDCH):
                m = 27 * kv + j * NDCH + dd
                W[:, m] = blk[:, :, dd].transpose(1, 0, 2)
                if kv == 0:   # V bias cancels in the distance
                    bias[:, m] = bsrc[dd * 128:(dd + 1) * 128] / TSS
    w_perm = to_f8(W)
    g_in = np.ascontiguousarray(ln_g.reshape(NDCH, 128).T * S_K).astype(BF16)
    b_in = np.ascontiguousarray(ln_b.reshape(NDCH, 128).T * S_K).astype(BF16)
    qind = np.zeros((128, 4, G_MAX), np.float32)
    for ch in range(4):
        for r in range(128):
            gr = 128 * ch + r
            if gr < G_MAX * T:
                qind[r, ch, gr // T] = 1.0
    sel_list, _, _ = _sel_pairs(counts)
    sel_in = to_f8(np.stack(sel_list, 0).transpose(1, 0, 2, 3))  # [128, nsel, 3, 128]

    in_maps = []
    for core in range(N_CORES):
        qs = q[core * NQL:(core + 1) * NQL]
        X = np.concatenate([s_sorted.reshape(-1, IN_DIM), qs.reshape(-1, IN_DIM)], 0)
        x_perm = np.ascontiguousarray(
            X.T.reshape(NKCH, 128, NX).transpose(1, 0, 2))
        in_maps.append({"x": to_f8(x_perm), "w": w_perm, "bias": bias,
                        "lng": g_in, "lnb": b_in, "qind": qind, "sel": sel_in})

    global _LAST_IN_MAPS
    _LAST_IN_MAPS = in_maps
    res = bass_utils.run_bass_kernel_spmd(nc, in_maps, core_ids=list(range(N_CORES)))
    return np.concatenate([res.results[i]["out"] for i in range(N_CORES)], 0)


_LAST_IN_MAPS = None


# revision 34
# speedup vs baseline: 1.0441x; 1.0441x over previous
"""Trainium2 Bass kernel for the CNN-TRX few-shot attention head.

Sharding: data-parallel over the 200 queries (25 per NeuronCore); support set
and weights replicated per core. v5: fp8(e4m3) DoubleRow matmuls everywhere,
PE-side V-path tuple gather via selection matmuls, natural-layout distances.

  1. Frame projections fp8 DoubleRow (contraction 2048 = 8 chunk pairs),
     K blocks first so the K-side DVE chain starts early; weights
     pre-scaled by 1024; K bias via per-partition activation bias; V bias
     dropped (it cancels exactly in ||q_v - proto||^2).
  2. K tuple gather (C(8,3)=56 triples) as 2-stage DVE column adds;
     V tuple gather on the PE: transpose frame projections to natural
     layout, then Sel.T @ frames with DoubleRow pairing tuple positions
     j0,j1 (+ plain j2) -- produces s_v / q_v directly in natural layout.
  3. Column LayerNorm of K: stats via 2-slot packed ones-matmuls, DVE
     apply writes fp8 (scale 4).
  4. scoresT = s_k^T q_k fp8 (4 DR pairs + 1 plain); exp with -ln(8) bias.
  5. P_nat[row,d] = exp^T s_v per class (DR row-chunk pairs); the s_v ones
     column gives S free; dist rows = ACT square-accum of (P*r - q_v) with
     r per-partition; logits via an indicator matmul.
"""

import math
from itertools import combinations

import ml_dtypes
import numpy as np

SEQ = 8
IN_DIM = 2048
OUT_DIM = 1152
TSS = 3
WAY = 5
N_SUPPORT = 25
N_QUERIES = 200
PE_SCALE = 0.1
LN_EPS = 1e-5
T = 56
N_CORES = 8
NQL = N_QUERIES // N_CORES      # queries per core (25)
G_SIZES = [8, 8, 9]             # query group sizes (sum = NQL)
G_MAX = max(G_SIZES)
C_ALLOC = 512
NKCH = IN_DIM // 128            # 16 contraction chunks
NDCH = OUT_DIM // 128           # 9
NMB = 6 * NDCH                  # 54 projection blocks
NX = SEQ * 2 * N_SUPPORT        # 400 frame columns per core
NFC = (NX + 127) // 128         # 4 frame-row chunks (natural layout)
PAIRS = [(t0, t1) for t0 in range(SEQ - 2) for t1 in range(t0 + 1, SEQ - 1)]
TUPLES = list(combinations(range(SEQ), TSS))
LN_CHUNK = 512
S_W = 1024.0                    # weight fp8 scale
S_K = 4.0                       # LN'd K fp8 scale
S_V = 4.0                       # V fp8 scale
EXP_SHIFT = math.log(8.0)       # exp output scale 1/8 (fp8 range)
SV_W = OUT_DIM + 16             # s_v width: 1152 d + ones col + pad
NSPL = [(0, 512), (512, 512), (1024, OUT_DIM - 1024)]
BF16 = ml_dtypes.bfloat16
F8 = ml_dtypes.float8_e4m3

_CACHE = {}


def _pos_encoding():
    pos = np.arange(SEQ, dtype=np.float32)[:, None]
    div = np.exp(np.arange(0, IN_DIM, 2, dtype=np.float32) * -(math.log(10000.0) / IN_DIM))
    pe = np.zeros((SEQ, IN_DIM), dtype=np.float32)
    pe[:, 0::2] = np.sin(pos * div) * PE_SCALE
    pe[:, 1::2] = np.cos(pos * div) * PE_SCALE
    return pe


def _class_layout(counts):
    offs, off = [], 0
    for c in range(WAY):
        offs.append(off)
        off += ((counts[c] * T + 127) // 128) * 128
    return offs, off


def _sel_pairs(counts):
    """Selection matrices for the PE-side V gather.

    Returns (sel_list, sup_chunks, qry_chunks):
      sel_list: list of [128, 3, 128] float32 0/1 matrices
      sup_chunks[tc] / qry_chunks[(g, tc)]: list of (fc, sel_index)
    Sel[frame_local_row, j, col] = 1 iff dst row (128*tc+col) is tuple a of
    item n and 8*n + TUPLES[a][j] == 128*fc + frame_local_row.
    """
    offs, nb_pad = _class_layout(counts)
    sel_list = []
    sup_chunks = {}

    def build(rows_of):  # rows_of: col -> (item n, tuple a) or None
        chunks = {}
        for col, na in rows_of.items():
            if na is None:
                continue
            n, a = na
            tc = col // 128
            for j in range(TSS):
                fr = 8 * n + TUPLES[a][j]
                fc = fr // 128
                key = (tc, fc)
                if key not in chunks:
                    chunks[key] = np.zeros((128, 3, 128), np.float32)
                chunks[key][fr - 128 * fc, j, col - 128 * tc] = 1.0
        out = {}
        for (tc, fc), mat in sorted(chunks.items()):
            sel_list.append(mat)
            out.setdefault(tc, []).append((fc, len(sel_list) - 1))
        return out

    rows = {}
    for c in range(WAY):
        n0 = int(np.sum(counts[:c]))
        for i in range(int(counts[c]) * T):
            rows[offs[c] + i] = (n0 + i // T, i % T)
    sup_chunks = build(rows)

    qry_chunks = {}
    it0 = N_SUPPORT
    for g, G in enumerate(G_SIZES):
        rows = {i: (it0 + i // T, i % T) for i in range(G * T)}
        for tc, lst in build(rows).items():
            qry_chunks[(g, tc)] = lst
        it0 += G
    return sel_list, sup_chunks, qry_chunks


def _build_kernel(counts, trivial_gb):
    import concourse.mybir as mybir
    import concourse.tile as tile
    from concourse import bacc
    from concourse.masks import make_identity

    f32 = mybir.dt.float32
    bf16 = mybir.dt.bfloat16
    fp8 = mybir.dt.float8e4
    AF = mybir.ActivationFunctionType
    ALU = mybir.AluOpType
    DR = mybir.MatmulPerfMode.DoubleRow
    offs, nb_pad = _class_layout(counts)
    nwch = nb_pad // 128
    inv_sqrt = 1.0 / math.sqrt(OUT_DIM)
    sel_list, sup_chunks, qry_chunks = _sel_pairs(counts)
    nsel = len(sel_list)

    nc = bacc.Bacc("TRN2", target_bir_lowering=False, debug=False,
                   enable_asserts=False, num_devices=N_CORES)

    x_d = nc.dram_tensor("x", [128, NKCH, NX], fp8, kind="ExternalInput").ap()
    w_d = nc.dram_tensor("w", [128, NMB, NKCH, 128], fp8, kind="ExternalInput").ap()
    bias_d = nc.dram_tensor("bias", [128, NMB], f32, kind="ExternalInput").ap()
    sel_d = nc.dram_tensor("sel", [128, nsel, 3, 128], fp8, kind="ExternalInput").ap()
    qind_d = nc.dram_tensor("qind", [128, 4, G_MAX], f32, kind="ExternalInput").ap()
    g_d = nc.dram_tensor("lng", [128, NDCH], bf16, kind="ExternalInput").ap()
    b_d = nc.dram_tensor("lnb", [128, NDCH], bf16, kind="ExternalInput").ap()
    out_d = nc.dram_tensor("out", [NQL, WAY], f32, kind="ExternalOutput").ap()

    with tile.TileContext(nc) as tc:
        with tc.tile_pool(name="big", bufs=1) as big, \
             tc.tile_pool(name="small", bufs=1) as small:
            # frame projections, fp8: K in T-layout (per tuple position j),
            # V transposed to natural layout nfv[frame_row, j, fc, d]
            f_k = [big.tile([128, NDCH, NX], fp8, name=f"fk{j}") for j in range(3)]
            nfv = big.tile([128, 3, NFC, SV_W], fp8, name="nfv")
            s_kT = big.tile([128, NDCH, nb_pad], fp8)       # LN'd support K * S_K
            s_v = big.tile([128, nwch, SV_W], fp8)          # support V * S_V, natural
            ones_bf = small.tile([128, 1], bf16)
            nc.vector.memset(ones_bf, 1.0)
            eps_sb = small.tile([1, 1], f32)
            nc.vector.memset(eps_sb, LN_EPS)
            expb_sb = small.tile([128, 1], f32)
            nc.vector.memset(expb_sb, -EXP_SHIFT)
            g_sb = small.tile([128, NDCH], bf16)
            b_sb = small.tile([128, NDCH], bf16)
            bias_sb = small.tile([128, NMB], f32)
            qind_sb = small.tile([128, 4, G_MAX], f32)
            nc.sync.dma_start(g_sb, g_d)
            nc.sync.dma_start(b_sb, b_d)
            nc.sync.dma_start(bias_sb, bias_d)
            nc.sync.dma_start(qind_sb, qind_d)
            logits5 = small.tile([WAY, NQL], f32)
            ident = small.tile([128, 128], bf16)
            make_identity(nc, ident)
            # last frame chunk holds only 16 rows -> zero the tail once
            nc.gpsimd.memset(nfv[:, :, NFC - 1, :], 0.0)

            # ---------- Phase 1: frame projections (fp8 DoubleRow) ----------
            # K blocks (m = j*9+dd) first, then V blocks (m = 27 + j*9+dd)
            sprep_cm = tc.tile_pool(name="sprep", bufs=2)
            pp_t_cm = tc.tile_pool(name="pp_t", bufs=4, space="PSUM")
            sprep = sprep_cm.__enter__()
            pp_t = pp_t_cm.__enter__()
            with tc.tile_pool(name="xt_pool", bufs=1) as xt_pool, \
                 tc.tile_pool(name="xw", bufs=3) as xw, \
                 tc.tile_pool(name="pp_proj", bufs=4, space="PSUM") as pp_proj:
                xt = xt_pool.tile([128, NKCH, NX], fp8)
                nc.sync.dma_start(xt, x_d)
                fv_ps = []          # V-block PSUM tiles, consumed by transposes
                for m in range(NMB):
                    kv, j, dd = m // 27, (m % 27) // 9, m % 9
                    wm = xw.tile([128, NKCH, 128], fp8, tag="wslab")
                    nc.sync.dma_start(wm, w_d[:, m])
                    ps = pp_proj.tile([128, NX], f32, tag="projps")
                    for k in range(NKCH // 2):
                        nc.tensor.matmul(ps, wm[:, 2 * k:2 * k + 2],
                                         xt[:, 2 * k:2 * k + 2],
                                         start=(k == 0), stop=(k == NKCH // 2 - 1),
                                         perf_mode=DR)
                    if kv == 0:
                        nc.scalar.activation(f_k[j][:, dd], ps, AF.Identity,
                                             bias=bias_sb[:, m:m + 1], scale=1.0 / S_W)
                    else:
                        # V: bf16 T-layout scratch (scaled S_V), then transpose
                        fvt = sprep.tile([128, NX], bf16, tag="fvt", name="fvt",
                                         bufs=4)
                        nc.scalar.activation(fvt, ps, AF.Copy, scale=S_V / S_W)
                        for fc in range(NFC):
                            cw = min(128, NX - fc * 128)
                            pst = pp_t.tile([128, 128], bf16, tag="tps")
                            nc.tensor.transpose(
                                pst[:cw], fvt[:, fc * 128:fc * 128 + cw], ident)
                            if dd % 2 == 0:
                                nc.vector.tensor_copy(
                                    nfv[:cw, j, fc, dd * 128:(dd + 1) * 128], pst[:cw])
                            else:
                                nc.scalar.activation(
                                    nfv[:cw, j, fc, dd * 128:(dd + 1) * 128],
                                    pst[:cw], AF.Copy)

            f_i = [fk.rearrange("p b (i s) -> p b i s", s=SEQ) for fk in f_k]

            def gather_k(dst4, items0, n_items, pool):
                """dst4 [128, 9, n_items, T] = tuple-gathered K projections."""
                isl = slice(items0, items0 + n_items)
                b0, b1, b2 = f_i
                p2 = pool.tile([128, NDCH, n_items, len(PAIRS)], bf16,
                               tag="pairs", name="p2")
                pi = 0
                for t0 in range(SEQ - 2):
                    run = SEQ - 2 - t0
                    a = b0[:, :, isl, t0:t0 + 1]
                    b = b1[:, :, isl, t0 + 1:t0 + 1 + run]
                    nc.vector.tensor_add(p2[:, :, :, pi:pi + run],
                                         a.to_broadcast(b.shape), b)
                    pi += run
                ai = 0
                for pi, (t0, t1) in enumerate(PAIRS):
                    run = SEQ - 1 - t1
                    a = p2[:, :, :, pi:pi + 1]
                    b = b2[:, :, isl, t1 + 1:t1 + 1 + run]
                    nc.vector.tensor_add(dst4[:, :, :, ai:ai + run],
                                         a.to_broadcast(b.shape), b)
                    ai += run

            def col_ln(raw, cols, out_q, pool, psum_pool):
                """Column-wise LayerNorm of raw [128, NDCH, cols] bf16
                (in place); writes fp8 out_q = LN(x)*S_K."""
                for c0 in range(0, cols, LN_CHUNK):
                    cw = min(LN_CHUNK, cols - c0)
                    r = raw[:, :, c0:c0 + cw]
                    o = out_q[:, :, c0:c0 + cw]
                    sq = pool.tile([128, NDCH, cw], bf16, tag="lnsq", name="lnsq", bufs=1)
                    nc.vector.tensor_mul(sq, r, r)
                    ps2 = psum_pool.tile([64, cw], f32, tag="lnps", name="lnps", bufs=1)
                    for k in range(NDCH):
                        nc.tensor.matmul(ps2[0:1], ones_bf, r[:, k],
                                         start=(k == 0), stop=(k == NDCH - 1),
                                         tile_position=(0, 0), skip_group_check=True)
                        nc.tensor.matmul(ps2[32:33], ones_bf, sq[:, k],
                                         start=(k == 0), stop=(k == NDCH - 1),
                                         tile_position=(0, 32), skip_group_check=True)
                    m_r = pool.tile([1, cw], f32, tag="lnm", name="lnm")
                    v_r = pool.tile([1, cw], f32, tag="lnv", name="lnv")
                    mm = pool.tile([1, cw], f32, tag="lnmm", name="lnmm")
                    nc.scalar.activation(m_r, ps2[0:1], AF.Copy, scale=1.0 / OUT_DIM)
                    nc.scalar.activation(v_r, ps2[32:33], AF.Copy, scale=1.0 / OUT_DIM)
                    nc.vector.tensor_mul(mm, m_r, m_r)
                    nc.vector.tensor_sub(v_r, v_r, mm)
                    nc.scalar.activation(v_r, v_r, AF.Sqrt, bias=eps_sb)
                    nc.vector.reciprocal(v_r, v_r)
                    m_h = pool.tile([1, cw], bf16, tag="lnmh", name="lnmh")
                    v_h = pool.tile([1, cw], bf16, tag="lnvh", name="lnvh")
                    nc.vector.tensor_copy(m_h, m_r)
                    if trivial_gb:
                        nc.vector.tensor_scalar(v_h, v_r, S_K, None, ALU.mult)
                    else:
                        nc.vector.tensor_copy(v_h, v_r)
                    m_b = pool.tile([128, cw], bf16, tag="lnmb", name="lnmb", bufs=1)
                    a_b = pool.tile([128, cw], bf16, tag="lnab", name="lnab", bufs=1)
                    nc.gpsimd.partition_broadcast(m_b, m_h)
                    nc.gpsimd.partition_broadcast(a_b, v_h)
                    mb3 = m_b[:, None, :].to_broadcast([128, NDCH, cw])
                    ab3 = a_b[:, None, :].to_broadcast([128, NDCH, cw])
                    nc.vector.tensor_sub(r, r, mb3)
                    if trivial_gb:
                        nc.vector.tensor_mul(o, r, ab3)
                    else:
                        nc.vector.tensor_mul(r, r, ab3)
                        for k in range(NDCH):
                            nc.vector.tensor_scalar(o[:, k], r[:, k],
                                                    g_sb[:, k:k + 1], b_sb[:, k:k + 1],
                                                    ALU.mult, ALU.add)

            def sel_gather(chunk_pairs, dst_fn, selpool, pspool, pstag):
                """dst[tc] rows = sum_j Sel_j^T nfv_j (tuple gather on PE)."""
                for tc, lst in sorted(chunk_pairs.items()):
                    sels = []
                    for fc, si in lst:
                        ssb = selpool.tile([128, 3, 128], fp8, tag="sel", name="sel")
                        nc.sync.dma_start(ssb, sel_d[:, si])
                        sels.append((fc, ssb))
                    for ni, (nlo, nw) in enumerate(NSPL):
                        ps = pspool.tile([128, 512], f32, tag=pstag)
                        for i, (fc, ssb) in enumerate(sels):
                            nc.tensor.matmul(ps[:, :nw], ssb[:, 0:2],
                                             nfv[:, 0:2, fc, nlo:nlo + nw],
                                             start=(i == 0), stop=False,
                                             perf_mode=DR)
                            nc.tensor.matmul(ps[:, :nw], ssb[:, 2],
                                             nfv[:, 2, fc, nlo:nlo + nw],
                                             start=False, stop=(i == len(sels) - 1))
                        dst_fn(tc, ni, nlo, nw, ps)

            # ---------- Phase 2: support-side tensors ----------
            max_ch = max((int(counts[c]) * T + 127) // 128 for c in range(WAY))
            with tc.tile_pool(name="pp_s", bufs=2, space="PSUM") as pp_s:
                start_item = 0
                for c in range(WAY):
                    n_c = int(counts[c])
                    rows = n_c * T
                    scratch = sprep.tile([128, NDCH, max_ch * 128], bf16,
                                         tag="skv", name="skv")
                    dst4 = scratch[:, :, :rows].rearrange("p b (n a) -> p b n a", a=T)
                    gather_k(dst4, start_item, n_c, sprep)
                    pad_lo = offs[c] + rows
                    pad_hi = offs[c + 1] if c + 1 < WAY else nb_pad
                    if pad_hi > pad_lo:
                        nc.gpsimd.memset(s_kT[:, :, pad_lo:pad_hi], 0.0)
                    col_ln(scratch[:, :, :rows], rows,
                           s_kT[:, :, offs[c]:offs[c] + rows], sprep, pp_s)
                    start_item += n_c
                # s_v ones col: 1 on real rows, 0 on pad rows
                for c in range(WAY):
                    rows = int(counts[c]) * T
                    wlo = offs[c] // 128
                    for w in range((rows + 127) // 128):
                        kreal = min(128, rows - w * 128)
                        if kreal < 128:
                            nc.gpsimd.memset(s_v[:, wlo + w, OUT_DIM:OUT_DIM + 1], 0.0)
                        nc.gpsimd.memset(s_v[:kreal, wlo + w, OUT_DIM:OUT_DIM + 1], 1.0)

                def sv_dst(tc, ni, nlo, nw, ps):
                    nc.scalar.activation(s_v[:, tc, nlo:nlo + nw], ps[:, :nw], AF.Copy)
                sel_gather(sup_chunks, sv_dst, sprep, pp_s, "selps")
            pp_t_cm.__exit__(None, None, None)
            sprep_cm.__exit__(None, None, None)

            # ---------- Phase 3: per-group query pipeline ----------
            with tc.tile_pool(name="grp", bufs=2) as grp, \
                 tc.tile_pool(name="rows", bufs=2) as rows_pool, \
                 tc.tile_pool(name="pp_sc", bufs=2, space="PSUM") as pp_sc, \
                 tc.tile_pool(name="pp_pr", bufs=3, space="PSUM") as pp_pr, \
                 tc.tile_pool(name="pp_row", bufs=2, space="PSUM") as pp_row:
                items0 = N_SUPPORT
                q_off = 0
                for g, G in enumerate(G_SIZES):
                    C = G * T
                    nmch = (C + 127) // 128
                    scr = grp.tile([128, NDCH, G_MAX, T], bf16, tag="qkv")
                    gather_k(scr[:, :, :G], items0, G, grp)
                    qk_raw = scr[:, :, :G].rearrange("p m q a -> p m (q a)")
                    qk3 = grp.tile([128, NDCH, C_ALLOC], fp8, tag="qk8")
                    col_ln(qk_raw, C, qk3[:, :, :C], grp, pp_row)

                    # q_v natural via PE sel-gather
                    qv_nat = grp.tile([128, 4, OUT_DIM], bf16, tag="qvn")

                    def qv_dst(tc, ni, nlo, nw, ps):
                        nc.scalar.activation(qv_nat[:, tc, nlo:nlo + nw],
                                             ps[:, :nw], AF.Copy)
                    sel_gather({tc: lst for (gg, tc), lst in qry_chunks.items()
                                if gg == g}, qv_dst, grp, pp_sc, "scps")

                    # scoresT (fp8: 4 DoubleRow pairs + 1 plain) + exp
                    exp_t = grp.tile([128, nwch, C_ALLOC], fp8, tag="exp")
                    for w in range(nwch):
                        ps = pp_sc.tile([128, C_ALLOC], f32, tag="scps")
                        for k in range(4):
                            nc.tensor.matmul(ps[:, :C],
                                             s_kT[:, 2 * k:2 * k + 2, w * 128:(w + 1) * 128],
                                             qk3[:, 2 * k:2 * k + 2, :C],
                                             start=(k == 0), stop=False, perf_mode=DR)
                        nc.tensor.matmul(ps[:, :C], s_kT[:, 8, w * 128:(w + 1) * 128],
                                         qk3[:, 8, :C], start=False, stop=True)
                        nc.scalar.activation(exp_t[:, w, :C], ps[:, :C], AF.Exp,
                                             scale=inv_sqrt / (S_K * S_K),
                                             bias=expb_sb)

                    dist5 = grp.tile([128, 4 * WAY], f32, tag="dist5")
                    nc.gpsimd.memset(dist5, 0.0)
                    scr_nat = grp.tile([128, OUT_DIM], bf16, tag="scrn")
                    ps_l = pp_row.tile([WAY, G_MAX], f32, tag="psl", bufs=1)
                    for c in range(WAY):
                        rows = int(counts[c]) * T
                        wlo = offs[c] // 128
                        nw_c = (rows + 127) // 128
                        np_pairs = nw_c // 2
                        for mc in range(nmch):
                            mw = min(128, C - mc * 128)
                            msl = slice(mc * 128, mc * 128 + mw)
                            psn = []
                            for ni in (2, 0, 1):
                                nlo, nw = NSPL[ni]
                                ps_p = pp_pr.tile([128, 512], f32, tag="prps")
                                psn.append((ni, ps_p))
                                nhi = nlo + nw + (1 if ni == 2 else 0)  # + S col
                                for wi in range(np_pairs):
                                    nc.tensor.matmul(
                                        ps_p[:mw, :nhi - nlo],
                                        exp_t[:, wlo + 2 * wi:wlo + 2 * wi + 2, msl],
                                        s_v[:, wlo + 2 * wi:wlo + 2 * wi + 2, nlo:nhi],
                                        start=(wi == 0),
                                        stop=(wi == np_pairs - 1 and nw_c % 2 == 0),
                                        perf_mode=DR)
                                if nw_c % 2 == 1:
                                    nc.tensor.matmul(
                                        ps_p[:mw, :nhi - nlo],
                                        exp_t[:, wlo + nw_c - 1, msl],
                                        s_v[:, wlo + nw_c - 1, nlo:nhi],
                                        start=(nw_c == 1), stop=True)
                                if ni == 2:
                                    rr = rows_pool.tile([128, 1], f32, tag="rr")
                                    nc.vector.reciprocal(
                                        rr[:mw], ps_p[:mw, NSPL[2][1]:NSPL[2][1] + 1])
                            # diff = P*r - q_v (both at 4x true scale)
                            for ni, ps_p in psn:
                                nlo, nw = NSPL[ni]
                                nc.vector.scalar_tensor_tensor(
                                    scr_nat[:mw, nlo:nlo + nw], ps_p[:mw, :nw],
                                    rr[:mw], qv_nat[:mw, mc, nlo:nlo + nw],
                                    ALU.mult, ALU.subtract)
                            nc.scalar.activation(
                                scr_nat[:mw], scr_nat[:mw], AF.Square,
                                accum_out=dist5[:mw, 4 * c + mc:4 * c + mc + 1])
                    # logits: indicator matmul sums dist rows per query
                    d54 = dist5.rearrange("p (c m) -> p c m", m=4)
                    for mc in range(nmch):
                        nc.tensor.matmul(ps_l[:, :G], d54[:, :, mc],
                                         qind_sb[:, mc, :G],
                                         start=(mc == 0), stop=(mc == nmch - 1))
                    nc.scalar.activation(logits5[:, q_off:q_off + G], ps_l[:, :G],
                                         AF.Copy, scale=-1.0 / (T * S_V * S_V))
                    items0 += G
                    q_off += G

            nc.sync.dma_start(out_d.rearrange("q c -> c q"), logits5)

    nc.compile()
    return nc


def kernel(support_set, support_labels, queries, k_w, k_b, v_w, v_b, ln_g, ln_b):
    import concourse.bass_utils as bass_utils

    support_set = np.asarray(support_set, dtype=np.float32)
    queries = np.asarray(queries, dtype=np.float32)
    labels = np.asarray(support_labels, dtype=np.int32)
    k_w = np.asarray(k_w, dtype=np.float32)
    v_w = np.asarray(v_w, dtype=np.float32)
    k_b = np.asarray(k_b, dtype=np.float32)
    v_b = np.asarray(v_b, dtype=np.float32)
    ln_g = np.asarray(ln_g, dtype=np.float32)
    ln_b = np.asarray(ln_b, dtype=np.float32)

    pe = _pos_encoding()
    s = support_set + pe[None]
    q = queries + pe[None]
    order = np.argsort(labels, kind="stable")
    counts = np.bincount(labels, minlength=WAY)
    s_sorted = s[order]
    trivial_gb = bool(np.all(ln_g == 1.0) and np.all(ln_b == 0.0))

    key = (tuple(int(x) for x in counts), trivial_gb)
    if key not in _CACHE:
        _CACHE[key] = _build_kernel(counts, trivial_gb)
    nc = _CACHE[key]

    def to_f8(x):
        return np.clip(x, -240.0, 240.0).astype(F8)

    # K blocks m = j*9+dd, V blocks m = 27 + j*9+dd; scaled by S_W
    W = np.zeros((128, NMB, NKCH, 128), np.float32)
    bias = np.zeros((128, NMB), np.float32)
    for kv, (wsrc, bsrc) in enumerate(((k_w, k_b), (v_w, v_b))):
        for j in range(TSS):
            blk = wsrc[j * IN_DIM:(j + 1) * IN_DIM] * S_W   # [2048, 1152]
            blk = blk.reshape(NKCH, 128, NDCH, 128)
            for dd in range(NDCH):
                m = 27 * kv + j * NDCH + dd
                W[:, m] = blk[:, :, dd].transpose(1, 0, 2)
                if kv == 0:   # V bias cancels in the distance
                    bias[:, m] = bsrc[dd * 128:(dd + 1) * 128] / TSS
    w_perm = to_f8(W)
    g_in = np.ascontiguousarray(ln_g.reshape(NDCH, 128).T * S_K).astype(BF16)
    b_in = np.ascontiguousarray(ln_b.reshape(NDCH, 128).T * S_K).astype(BF16)
    qind = np.zeros((128, 4, G_MAX), np.float32)
    for ch in range(4):
        for r in range(128):
            gr = 128 * ch + r
            if gr < G_MAX * T:
                qind[r, ch, gr // T] = 1.0
    sel_list, _, _ = _sel_pairs(counts)
    sel_in = to_f8(np.stack(sel_list, 0).transpose(1, 0, 2, 3))  # [128, nsel, 3, 128]

    in_maps = []
    for core in range(N_CORES):
        qs = q[core * NQL:(core + 1) * NQL]
        X = np.concatenate([s_sorted.reshape(-1, IN_DIM), qs.reshape(-1, IN_DIM)], 0)
        x_perm = np.ascontiguousarray(
            X.T.reshape(NKCH, 128, NX).transpose(1, 0, 2))
        in_maps.append({"x": to_f8(x_perm), "w": w_perm, "bias": bias,
                        "lng": g_in, "lnb": b_in, "qind": qind, "sel": sel_in})

    global _LAST_IN_MAPS
    _LAST_IN_MAPS = in_maps
    res = bass_utils.run_bass_kernel_spmd(nc, in_maps, core_ids=list(range(N_CORES)))
    return np.concatenate([res.results[i]["out"] for i in range(N_CORES)], 0)


_LAST_IN_MAPS = None
